# revision 32
# baseline (speedup 1.0000x reference)
"""AdaptiveFourier1d TRN2 kernel: y = irfft(MLP_freq(rfft(x))) + x.

Data-parallel over batch: 8 cores x 2 batch elements each. Per core 16
units of (96 ch, 4096). Radix-64^2 Cooley-Tukey FFT on TensorE; transposes
via DVE StreamTranspose (32x32 blocks) + block-permute DMAs; twiddles as
mixed-base DVE ops reading PSUM; MLP contracts channels.

Self-contained: all shapes hardcoded; constant tables passed as extra
dram parameters computed here with numpy.
"""
import os
import sys

sys.path.insert(0, "/opt/trn_rl_repo")

import numpy as np
import ml_dtypes

import concourse.bass as bass
import concourse.mybir as mybir
import concourse.tile as tile
from concourse.vector_clock import ScopedClock
from concourse.bass_utils import run_bass_kernel_spmd

bf16 = mybir.dt.bfloat16
f32 = mybir.dt.float32
RELU = mybir.ActivationFunctionType.Relu
COPY = mybir.ActivationFunctionType.Copy
MUL = mybir.AluOpType.mult
ADD = mybir.AluOpType.add
SUB = mybir.AluOpType.subtract

B, C, N = 16, 768, 4096
NB, BLK = 8, 96
N1 = N2 = 64
K2X = 33
LAM = 0.01
NCORES = 8
BLOC = B // NCORES
UNITS = BLOC * NB
FREE = BLK * N2          # 6144
CH = 512
TCH = 1024               # fwd twiddle chunk: 16 c x 64 n2
ICH = 384                # inv twiddle chunk: 4 k1 x 96 c'
SLAB = 8                 # MLP k1 slab

N_UNITS = int(os.environ.get("N_UNITS", UNITS))
STOP_AFTER = os.environ.get("STOP_AFTER", "")


def _patched_drain_and_barrier(self, tick_clock, wait_clock):
    nc = self.nc
    nops = [nc.sync.nop(nofuse=True) for _ in range(64)]
    drain_inst = nc.sync.drain()
    wait_clock.add_sem_waits(
        drain_inst.ins, ScopedClock({None: tick_clock.global_clock})
    )
    si = drain_inst.ins.sync_info
    waits = list(si.on_wait) if si and si.on_wait else []
    if len(waits) > 1:
        si.on_wait = [waits[0]]
        for w, nop in zip(waits[1:], nops):
            nsi = nop.ins.sync_info
            if nsi is None:
                nop.ins.sync_info = mybir.SyncInfo(on_wait=[w], on_update=[])
            else:
                nsi.on_wait = [w]
    nc.all_engine_barrier()
    popped = nc._tile_sem_poison_stack.pop()
    assert popped is self._sem_poison
    nc.clear_and_free_semaphores(list(self.sems.allocated().values()))
    nc.all_engine_barrier()


tile.TileContext._drain_and_barrier = _patched_drain_and_barrier

# walrus in this toolchain rejects >2 sync waits per instruction; split extras
# onto same-engine nops committed just before the instruction.
MAXW = 1
_orig_commit = tile.TileContext._commit_instruction


def _commit_split(self, inst, lazy_reg_writes=True):
    si = getattr(inst, "sync_info", None)
    if (si is not None and si.on_wait and len(si.on_wait) > MAXW
            and inst.engine != mybir.EngineType.Unassigned):
        waits = list(si.on_wait)
        si.on_wait = waits[-MAXW:]
        extra = waits[:-MAXW]
        eng = self.nc.engines[inst.engine]
        for i in range(0, len(extra), MAXW):
            nop = eng.nop(nofuse=True)
            nop.ins.sync_info = mybir.SyncInfo(on_wait=extra[i:i + MAXW], on_update=[])
    return _orig_commit(self, inst, lazy_reg_writes)


tile.TileContext._commit_instruction = _commit_split


def make_tables():
    t = {}
    n1 = np.arange(N1)
    k1 = np.arange(N1)
    n2 = np.arange(N2)
    k2 = np.arange(K2X)

    a = 2 * np.pi * np.outer(n1, k1) / N1
    t["s1w"] = np.concatenate([np.cos(a), -np.sin(a)], axis=1) / 64.0  # (64,128)

    ang = 2 * np.pi * np.outer(k1, n2) / N          # T[k1, n2] = e^{-i ang}
    tt1 = np.tile(np.cos(ang), (1, BLK))            # (64, FREE) c-outer n2-inner
    tsn = np.tile(np.sin(ang), (1, BLK))
    t["tt1"] = np.concatenate([tt1, tt1], axis=0)   # (128, FREE) for full-width ops
    t["tts"] = np.concatenate([-tsn, tsn], axis=0)  # rows0:-sin (for bim), rows1:+sin (bre)

    o_ang = 2 * np.pi * np.outer(n2, k2) / N2       # Omega = e^{-i o_ang}
    Ore, Oim = np.cos(o_ang), -np.sin(o_ang)
    z = np.zeros((N2, 96 - 2 * K2X))
    top = np.concatenate([Ore, Oim, z], axis=1)
    bot = np.concatenate([-Oim, Ore, z], axis=1)
    t["s2w"] = np.concatenate([top, bot], axis=0)   # (128, 96)

    ia_ang = 2 * np.pi * np.outer(k2, n2) / N2      # e^{+i}
    iare = np.concatenate([2 * np.cos(ia_ang), 2 * np.sin(ia_ang)], axis=1)
    iaim = np.concatenate([-2 * np.sin(ia_ang), 2 * np.cos(ia_ang)], axis=1)
    t["iaw"] = np.concatenate([iare, iaim, np.zeros((96 - 2 * K2X, 128))], axis=0)

    it_ang = 2 * np.pi * np.outer(n2, k1) / N       # T'[n2',k1] = e^{+i}
    # chunk order: (k1-group-of-4 outer, c middle, k1-within inner)
    def ck(m):
        return np.broadcast_to(
            m.reshape(N1, 16, 1, 4), (N1, 16, BLK, 4)).reshape(N1, FREE)
    itt1 = ck(np.cos(it_ang))
    itsn = ck(np.sin(it_ang))
    t["itt1"] = np.concatenate([itt1, itt1], axis=0)
    t["itts"] = np.concatenate([itsn, -itsn], axis=0)  # rows0:+sin (gim), rows1:-sin (gre)

    ic_ang = 2 * np.pi * np.outer(k1, n1) / N1      # e^{+i}
    t["icw"] = np.concatenate([np.cos(ic_ang), -np.sin(ic_ang)], axis=0) / 64.0

    return {k: np.ascontiguousarray(v, dtype=np.float32) for k, v in t.items()}


TBL_SHAPES = {"s1w": (64, 128), "tt1": (128, FREE), "tts": (128, FREE),
              "s2w": (128, 96), "iaw": (96, 128),
              "itt1": (128, FREE), "itts": (128, FREE), "icw": (128, 64)}


def build_nc():
    nc = bass.Bass()
    x_ext = nc.declare_dram_parameter("x", [BLOC * C, N], f32, isOutput=False)
    out_ext = nc.declare_dram_parameter("out", [BLOC * C, N], f32, isOutput=True)
    wext = {}
    for nm in ["w1_re", "w1_im", "w2_re", "w2_im"]:
        wext[nm] = nc.declare_dram_parameter(nm, [NB, BLK, BLK], f32, isOutput=False)
    for nm in ["b1_re", "b1_im", "b2_re", "b2_im"]:
        wext[nm] = nc.declare_dram_parameter(nm, [NB, BLK], f32, isOutput=False)
    text = {nm: nc.declare_dram_parameter(nm, list(shp), f32, isOutput=False)
            for nm, shp in TBL_SHAPES.items()}

    with tile.TileContext(nc) as tc:
        with tc.tile_pool(name="cst", bufs=1) as cst, \
             tc.tile_pool(name="work", bufs=1) as pool, \
             tc.tile_pool(name="psbig", bufs=2, space="PSUM") as psB, \
             tc.tile_pool(name="psa", bufs=2, space="PSUM") as psA, \
             tc.tile_pool(name="psb", bufs=2, space="PSUM") as psD:

            # ---- constants ----
            ct = {}
            for nm in TBL_SHAPES:
                ct[nm] = cst.tile(list(TBL_SHAPES[nm]), bf16, tag=nm, name=nm)
                nc.gpsimd.dma_start(out=ct[nm][:], in_=text[nm][:])
            wt = {}
            for nm, src in [("w1re", "w1_re"), ("w1im", "w1_im"),
                            ("w2re", "w2_re"), ("w2im", "w2_im")]:
                wt[nm] = cst.tile([BLK, NB, BLK], bf16, tag=nm, name=nm)
                nc.gpsimd.dma_start(out=wt[nm][:], in_=wext[src].rearrange("b c o -> c b o"))
            w1imN = cst.tile([BLK, NB, BLK], bf16, tag="w1imN")
            nc.vector.tensor_scalar_mul(w1imN[:], wt["w1im"][:], -1.0)
            w2imN = cst.tile([BLK, NB, BLK], bf16, tag="w2imN")
            nc.vector.tensor_scalar_mul(w2imN[:], wt["w2im"][:], -1.0)
            bia = {}
            for nm, src in [("b1re", "b1_re"), ("b1im", "b1_im"),
                            ("b2re", "b2_re"), ("b2im", "b2_im")]:
                bia[nm] = cst.tile([BLK, NB], f32, tag=nm, name=nm)
                nc.gpsimd.dma_start(out=bia[nm][:], in_=wext[src].rearrange("b c -> c b"))
            b2v = {}
            for nm in ["b2re", "b2im"]:
                m = cst.tile([BLK, NB], f32, tag=f"{nm}_m", name=f"{nm}_m")   # b2 - lam
                nc.vector.tensor_scalar_add(m[:], bia[nm][:], -LAM)
                n_ = cst.tile([BLK, NB], f32, tag=f"{nm}_n", name=f"{nm}_n")  # -b2 - lam
                nc.vector.tensor_scalar(out=n_[:], in0=bia[nm][:], scalar1=-1.0,
                                        scalar2=-LAM, op0=MUL, op1=ADD)
                b2v[nm] = (m, n_)

            dma_engs = [nc.sync, nc.scalar, nc.gpsimd]

            def xpose(in_ap, out_t, P, M, Q, tag, eng=None, qoff=0):
                """in (P p; M, Q f) -> out (Q p; M, P f). P, Q mult of 32."""
                st = pool.tile([P, M, Q], bf16, tag=tag, name=tag)
                nc.vector.transpose(out=st[:], in_=in_ap)
                for pb in range(P // 32):
                    for qb in range(Q // 32):
                        xpose.rr = getattr(xpose, "rr", 0) + 1
                        e = dma_engs[xpose.rr % 3]
                        e.dma_start(
                            out=out_t[qoff + qb * 32: qoff + qb * 32 + 32, :,
                                      pb * 32: pb * 32 + 32],
                            in_=st[pb * 32: pb * 32 + 32, :, qb * 32: qb * 32 + 32])

            for u in range(N_UNITS):
                blk = u % NB
                bs = slice(blk, blk + 1)
                rows = slice(u * BLK, (u + 1) * BLK)
                xr = x_ext[rows].rearrange("c (a b) -> a c b", a=N1)

                # ---- P1: load, S1, fwd twiddle ----
                xt = pool.tile([64, BLK, N2], bf16, tag=f"xt{u % 2}")
                nc.gpsimd.dma_start(out=xt[:], in_=xr)
                xtf = xt[:].rearrange("p c b -> p (c b)")
                bduo = pool.tile([64, BLK, 128], bf16, tag="duo")
                for j in range(FREE // TCH):
                    a_ps = psB.tile([128, TCH], f32, tag="ps128")
                    for m in range(TCH // CH):
                        pc = slice(m * CH, (m + 1) * CH)
                        col = slice(j * TCH + m * CH, j * TCH + (m + 1) * CH)
                        nc.tensor.matmul(out=a_ps[:, pc], lhsT=ct["s1w"][:],
                                         rhs=xtf[:, col], start=True, stop=True)
                    sl = slice(j * TCH, (j + 1) * TCH)
                    abf = pool.tile([128, TCH], bf16, tag=f"abf{j % 2}")
                    nc.scalar.activation(out=abf[:], in_=a_ps[:], func=COPY)
                    uu = pool.tile([128, TCH], bf16, tag="uuF")
                    vv = pool.tile([128, TCH], bf16, tag="vvF")
                    nc.vector.tensor_tensor(out=uu[:], in0=abf[:], in1=ct["tt1"][:, sl], op=MUL)
                    nc.vector.tensor_tensor(out=vv[0:64, :], in0=abf[64:128, :],
                                            in1=ct["tts"][64:128, sl], op=MUL)
                    nc.vector.tensor_tensor(out=vv[64:128, :], in0=abf[0:64, :],
                                            in1=ct["tts"][0:64, sl], op=MUL)
                    cs = slice((j * TCH) // N2, ((j + 1) * TCH) // N2)
                    bre = bduo[:, cs, 0:64]
                    bim = bduo[:, cs, 64:128]
                    nc.vector.tensor_tensor(out=bre, in0=uu[0:64, :], in1=vv[0:64, :], op=ADD)
                    nc.vector.tensor_tensor(out=bim, in0=uu[64:128, :], in1=vv[64:128, :], op=ADD)

                def dump64(t):  # (64, 96, 64)-shaped bf16 tile -> out rows
                    nc.gpsimd.dma_start(out=out_ext[rows].rearrange("c (a b) -> a c b", a=N1), in_=t)

                def dump96(t):  # (96, X>=4096 flat) bf16 tile -> out rows
                    nc.gpsimd.dma_start(out=out_ext[rows], in_=t)

                if STOP_AFTER == "xt":
                    dump64(xt[:]); continue
                if STOP_AFTER == "bduo":
                    dump64(bduo[:, :, 0:64]); continue
                if STOP_AFTER == "bduo_im":
                    dump64(bduo[:, :, 64:128]); continue

                # ---- T1 (re/im halves; DVE on one half overlaps DMAs of other) ----
                bt = pool.tile([128, BLK, 64], bf16, tag=f"bt{u % 2}")
                xpose(bduo[:, :, 0:64], bt, 64, BLK, 64, "stA", eng=nc.sync, qoff=0)
                xpose(bduo[:, :, 64:128], bt, 64, BLK, 64, "stB", eng=nc.scalar, qoff=64)

                if STOP_AFTER == "bt":
                    dump64(bt[0:64]); continue
                if STOP_AFTER == "bt_im":
                    dump64(bt[64:128]); continue

                # ---- S2 + evict: (k1,c)-ordered chunks, contiguous act dst ----
                xf = pool.tile([96, 64, BLK], bf16, tag=f"mid1{u % 2}")  # (k2x; k1, c)
                SCH = 4 * BLK  # 4 k1 x 96 c
                for j in range(FREE // SCH):
                    s_ps = psA.tile([96, SCH], f32, tag="psa")
                    rhs = bt[:, :, j * 4:(j + 1) * 4].rearrange("p c k -> p k c")
                    nc.tensor.matmul(out=s_ps[:], lhsT=ct["s2w"][:],
                                     rhs=rhs, start=True, stop=True)
                    nc.scalar.activation(out=xf[:, j * 4:(j + 1) * 4, :], in_=s_ps[:],
                                         func=COPY)

                if STOP_AFTER == "xf":
                    dump96(xf[:].rearrange("p a b -> p (a b)")[:, 0:4096]); continue

                # ---- T2 (direct contiguous read) ----
                xft = pool.tile([96, 64, 96], bf16, tag="mid2")  # (c; k1, k2x96)
                xpose(xf[:], xft, 96, 64, 96, "stA", eng=nc.gpsimd)

                if STOP_AFTER == "xft":
                    dump96(xft[:].rearrange("p a b -> p (a b)")[:, 0:4096]); continue

                # ---- MLP ----
                ot = pool.tile([96, 64, 96], bf16, tag=f"mid1{u % 2}")
                for s in range(64 // SLAB):
                    ksl = slice(s * SLAB, (s + 1) * SLAB)
                    rre = xft[:, ksl, 0:33]
                    rim = xft[:, ksl, 33:66]
                    hre_ps = psA.tile([96, SLAB * 33], f32, tag="psa")
                    him_ps = psD.tile([96, SLAB * 33], f32, tag="psb")
                    nc.tensor.matmul(out=hre_ps[:], lhsT=wt["w1re"][:, blk], rhs=rre,
                                     start=True, stop=False, skip_group_check=True)
                    nc.tensor.matmul(out=him_ps[:], lhsT=wt["w1re"][:, blk], rhs=rim,
                                     start=True, stop=False, skip_group_check=True)
                    nc.tensor.matmul(out=hre_ps[:], lhsT=w1imN[:, blk], rhs=rim,
                                     start=False, stop=True, skip_group_check=True)
                    nc.tensor.matmul(out=him_ps[:], lhsT=wt["w1im"][:, blk], rhs=rre,
                                     start=False, stop=True, skip_group_check=True)
                    hre = pool.tile([96, SLAB * 33], bf16, tag=f"hre_s{s % 2}")
                    him = pool.tile([96, SLAB * 33], bf16, tag=f"him_s{s % 2}")
                    nc.scalar.activation(out=hre[:], in_=hre_ps[:], func=RELU,
                                         bias=bia["b1re"][:, bs], scale=1.0)
                    nc.scalar.activation(out=him[:], in_=him_ps[:], func=RELU,
                                         bias=bia["b1im"][:, bs], scale=1.0)
                    ore_ps = psA.tile([96, SLAB * 33], f32, tag="psa")
                    oim_ps = psD.tile([96, SLAB * 33], f32, tag="psb")
                    nc.tensor.matmul(out=ore_ps[:], lhsT=wt["w2re"][:, blk], rhs=hre[:],
                                     start=True, stop=False, skip_group_check=True)
                    nc.tensor.matmul(out=oim_ps[:], lhsT=wt["w2re"][:, blk], rhs=him[:],
                                     start=True, stop=False, skip_group_check=True)
                    nc.tensor.matmul(out=ore_ps[:], lhsT=w2imN[:, blk], rhs=him[:],
                                     start=False, stop=True, skip_group_check=True)
                    nc.tensor.matmul(out=oim_ps[:], lhsT=wt["w2im"][:, blk], rhs=hre[:],
                                     start=False, stop=True, skip_group_check=True)
                    for ps_, nm, qoff in [(ore_ps, "b2re", 0), (oim_ps, "b2im", 33)]:
                        bm, bn = b2v[nm]
                        t1_ = pool.tile([96, SLAB * 33], bf16, tag=f"ss1{s % 2}")
                        t2_ = pool.tile([96, SLAB * 33], bf16, tag=f"ss2{s % 2}")
                        nc.scalar.activation(out=t1_[:], in_=ps_[:], func=RELU,
                                             bias=bm[:, bs], scale=1.0)
                        nc.scalar.activation(out=t2_[:], in_=ps_[:], func=RELU,
                                             bias=bn[:, bs], scale=-1.0)
                        dst = ot[:, ksl, qoff:qoff + 33]
                        nc.vector.tensor_tensor(out=dst, in0=t1_[:], in1=t2_[:], op=SUB)

                # fixups: DC & Nyquist bins g=1 (halve); junk bins (k2=32,k1>0) zero
                for col in (0, 33, 32, 65):
                    nc.vector.tensor_scalar_mul(ot[:, 0, col:col + 1], ot[:, 0, col:col + 1], 0.5)
                nc.vector.memset(ot[:, 1:64, 32:33], 0.0)
                nc.vector.memset(ot[:, 1:64, 65:66], 0.0)

                if STOP_AFTER == "ot":
                    dump96(ot[:].rearrange("p a b -> p (a b)")[:, 0:4096]); continue

                # ---- T3 ----
                ott = pool.tile([96, 64, 96], bf16, tag="mid2")
                xpose(ot[:], ott, 96, 64, 96, "stB", eng=nc.gpsimd)

                if STOP_AFTER == "ott":
                    dump96(ott[:].rearrange("p a b -> p (a b)")[:, 0:4096]); continue

                # ---- iA + inverse twiddle ((c,k1)-ordered, contiguous dst) ----
                gduo = pool.tile([64, BLK, 128], bf16, tag="duo")  # (n2; c, k1pair)
                for j in range(FREE // ICH):
                    e_ps = psD.tile([128, ICH], f32, tag="psb")
                    rhs = ott[:, j * 4:(j + 1) * 4, :].rearrange("p k c -> p c k")
                    nc.tensor.matmul(out=e_ps[:], lhsT=ct["iaw"][:],
                                     rhs=rhs, start=True, stop=True)
                    sl = slice(j * ICH, (j + 1) * ICH)
                    ebf = pool.tile([128, ICH], bf16, tag=f"abf{j % 2}")
                    nc.scalar.activation(out=ebf[:], in_=e_ps[:], func=COPY)
                    uu = pool.tile([128, ICH], bf16, tag="uuF")
                    vv = pool.tile([128, ICH], bf16, tag="vvF")
                    nc.vector.tensor_tensor(out=uu[:], in0=ebf[:], in1=ct["itt1"][:, sl], op=MUL)
                    nc.vector.tensor_tensor(out=vv[0:64, :], in0=ebf[64:128, :],
                                            in1=ct["itts"][64:128, sl], op=MUL)
                    nc.vector.tensor_tensor(out=vv[64:128, :], in0=ebf[0:64, :],
                                            in1=ct["itts"][0:64, sl], op=MUL)
                    k0 = (j * ICH) // BLK
                    kn = ICH // BLK
                    nc.vector.tensor_tensor(out=gduo[:, :, k0:k0 + kn], in0=uu[0:64, :], in1=vv[0:64, :], op=ADD)
                    nc.vector.tensor_tensor(out=gduo[:, :, 64 + k0:64 + k0 + kn], in0=uu[64:128, :], in1=vv[64:128, :], op=ADD)

                if STOP_AFTER == "gduo":
                    dump64(gduo[:, :, 0:64]); continue

                # ---- T4 (re/im halves, direct contiguous reads) ----
                gt = pool.tile([128, BLK, 64], bf16, tag=f"bt{u % 2}")
                xpose(gduo[:, :, 0:64], gt, 64, BLK, 64, "stA", eng=nc.sync, qoff=0)
                xpose(gduo[:, :, 64:128], gt, 64, BLK, 64, "stB", eng=nc.scalar, qoff=64)

                if STOP_AFTER == "gt":
                    dump64(gt[0:64]); continue

                # ---- iC + residual (from xt) + store ----
                ybf = pool.tile([64, BLK, N2], bf16, tag="mid2")
                gtf = gt[:].rearrange("p c k -> p (c k)")
                xresf = xtf
                ybff = ybf[:].rearrange("p c b -> p (c b)")
                for j in range(FREE // CH):
                    y_ps = psA.tile([64, CH], f32, tag="psa")
                    nc.tensor.matmul(out=y_ps[:], lhsT=ct["icw"][:],
                                     rhs=gtf[:, j * CH:(j + 1) * CH], start=True, stop=True)
                    nc.vector.tensor_tensor(out=ybff[:, j * CH:(j + 1) * CH],
                                            in0=y_ps[:], in1=xresf[:, j * CH:(j + 1) * CH], op=ADD)
                orow = out_ext[rows].rearrange("c (a b) -> a c b", a=N1)
                nc.gpsimd.dma_start(out=orow, in_=ybf[:])

    return nc


_CACHE = {}


def kernel(**inputs):
    x = np.asarray(inputs["x"], dtype=np.float32)
    tables = make_tables()
    if "nc" not in _CACHE:
        _CACHE["nc"] = build_nc()
    nc = _CACHE["nc"]
    xs = x.reshape(NCORES, BLOC * C, N)
    wkeys = ["w1_re", "w1_im", "w2_re", "w2_im", "b1_re", "b1_im", "b2_re", "b2_im"]
    in_maps = []
    for cix in range(NCORES):
        m = {"x": np.ascontiguousarray(xs[cix])}
        for k in wkeys:
            m[k] = np.asarray(inputs[k], dtype=np.float32)
        m.update(tables)
        in_maps.append(m)
    res = run_bass_kernel_spmd(nc, in_maps, core_ids=list(range(NCORES)))
    _CACHE["last_res"] = res
    out = np.stack([res.results[i]["out"] for i in range(NCORES)], axis=0)
    return out.reshape(B, C, N)


if __name__ == "__main__":
    rng = np.random.default_rng(0)
    ins = {"x": rng.standard_normal((B, C, N), dtype=np.float32)}
    for k in ["w1_re", "w1_im", "w2_re", "w2_im"]:
        ins[k] = 0.02 * rng.random((NB, BLK, BLK), dtype=np.float32)
    for k in ["b1_re", "b1_im", "b2_re", "b2_im"]:
        ins[k] = 0.02 * rng.random((NB, BLK), dtype=np.float32)
    y = kernel(**ins)
    print("kernel ran, out shape", y.shape)



# revision 34
# speedup vs baseline: 1.0150x; 1.0150x over previous
"""AdaptiveFourier1d TRN2 kernel: y = irfft(MLP_freq(rfft(x))) + x.

Data-parallel over batch: 8 cores x 2 batch elements each. Per core 16
units of (96 ch, 4096). Radix-64^2 Cooley-Tukey FFT on TensorE; transposes
via DVE StreamTranspose (32x32 blocks) + block-permute DMAs; twiddles as
mixed-base DVE ops reading PSUM; MLP contracts channels.

Self-contained: all shapes hardcoded; constant tables passed as extra
dram parameters computed here with numpy.
"""
import os
import sys

sys.path.insert(0, "/opt/trn_rl_repo")

import numpy as np
import ml_dtypes

import concourse.bass as bass
import concourse.mybir as mybir
import concourse.tile as tile
from concourse.vector_clock import ScopedClock
from concourse.bass_utils import run_bass_kernel_spmd

bf16 = mybir.dt.bfloat16
f32 = mybir.dt.float32
RELU = mybir.ActivationFunctionType.Relu
COPY = mybir.ActivationFunctionType.Copy
MUL = mybir.AluOpType.mult
ADD = mybir.AluOpType.add
SUB = mybir.AluOpType.subtract

B, C, N = 16, 768, 4096
NB, BLK = 8, 96
N1 = N2 = 64
K2X = 33
LAM = 0.01
NCORES = 8
BLOC = B // NCORES
UNITS = BLOC * NB
FREE = BLK * N2          # 6144
CH = 512
TCH = 1024               # fwd twiddle chunk: 16 c x 64 n2
ICH = 384                # inv twiddle chunk: 4 k1 x 96 c'
SLAB = 8                 # MLP k1 slab

N_UNITS = int(os.environ.get("N_UNITS", UNITS))
STOP_AFTER = os.environ.get("STOP_AFTER", "")


def _patched_drain_and_barrier(self, tick_clock, wait_clock):
    nc = self.nc
    nops = [nc.sync.nop(nofuse=True) for _ in range(64)]
    drain_inst = nc.sync.drain()
    wait_clock.add_sem_waits(
        drain_inst.ins, ScopedClock({None: tick_clock.global_clock})
    )
    si = drain_inst.ins.sync_info
    waits = list(si.on_wait) if si and si.on_wait else []
    if len(waits) > 1:
        si.on_wait = [waits[0]]
        for w, nop in zip(waits[1:], nops):
            nsi = nop.ins.sync_info
            if nsi is None:
                nop.ins.sync_info = mybir.SyncInfo(on_wait=[w], on_update=[])
            else:
                nsi.on_wait = [w]
    nc.all_engine_barrier()
    popped = nc._tile_sem_poison_stack.pop()
    assert popped is self._sem_poison
    nc.clear_and_free_semaphores(list(self.sems.allocated().values()))
    nc.all_engine_barrier()


tile.TileContext._drain_and_barrier = _patched_drain_and_barrier

# walrus in this toolchain rejects >2 sync waits per instruction; split extras
# onto same-engine nops committed just before the instruction.
MAXW = 1
_orig_commit = tile.TileContext._commit_instruction


def _commit_split(self, inst, lazy_reg_writes=True):
    si = getattr(inst, "sync_info", None)
    if (si is not None and si.on_wait and len(si.on_wait) > MAXW
            and inst.engine != mybir.EngineType.Unassigned):
        waits = list(si.on_wait)
        si.on_wait = waits[-MAXW:]
        extra = waits[:-MAXW]
        eng = self.nc.engines[inst.engine]
        for i in range(0, len(extra), MAXW):
            nop = eng.nop(nofuse=True)
            nop.ins.sync_info = mybir.SyncInfo(on_wait=extra[i:i + MAXW], on_update=[])
    return _orig_commit(self, inst, lazy_reg_writes)


tile.TileContext._commit_instruction = _commit_split


def make_tables():
    t = {}
    n1 = np.arange(N1)
    k1 = np.arange(N1)
    n2 = np.arange(N2)
    k2 = np.arange(K2X)

    a = 2 * np.pi * np.outer(n1, k1) / N1
    t["s1w"] = np.concatenate([np.cos(a), -np.sin(a)], axis=1) / 64.0  # (64,128)

    ang = 2 * np.pi * np.outer(k1, n2) / N          # T[k1, n2] = e^{-i ang}
    tt1 = np.tile(np.cos(ang), (1, BLK))            # (64, FREE) c-outer n2-inner
    tsn = np.tile(np.sin(ang), (1, BLK))
    t["tt1"] = np.concatenate([tt1, tt1], axis=0)   # (128, FREE) for full-width ops
    t["tts"] = np.concatenate([-tsn, tsn], axis=0)  # rows0:-sin (for bim), rows1:+sin (bre)

    o_ang = 2 * np.pi * np.outer(n2, k2) / N2       # Omega = e^{-i o_ang}
    Ore, Oim = np.cos(o_ang), -np.sin(o_ang)
    z = np.zeros((N2, 96 - 2 * K2X))
    top = np.concatenate([Ore, Oim, z], axis=1)
    bot = np.concatenate([-Oim, Ore, z], axis=1)
    t["s2w"] = np.concatenate([top, bot], axis=0)   # (128, 96)

    ia_ang = 2 * np.pi * np.outer(k2, n2) / N2      # e^{+i}
    iare = np.concatenate([2 * np.cos(ia_ang), 2 * np.sin(ia_ang)], axis=1)
    iaim = np.concatenate([-2 * np.sin(ia_ang), 2 * np.cos(ia_ang)], axis=1)
    t["iaw"] = np.concatenate([iare, iaim, np.zeros((96 - 2 * K2X, 128))], axis=0)

    it_ang = 2 * np.pi * np.outer(n2, k1) / N       # T'[n2',k1] = e^{+i}
    # chunk order: (k1-group-of-4 outer, c middle, k1-within inner)
    def ck(m):
        return np.broadcast_to(
            m.reshape(N1, 16, 1, 4), (N1, 16, BLK, 4)).reshape(N1, FREE)
    itt1 = ck(np.cos(it_ang))
    itsn = ck(np.sin(it_ang))
    t["itt1"] = np.concatenate([itt1, itt1], axis=0)
    t["itts"] = np.concatenate([itsn, -itsn], axis=0)  # rows0:+sin (gim), rows1:-sin (gre)

    ic_ang = 2 * np.pi * np.outer(k1, n1) / N1      # e^{+i}
    t["icw"] = np.concatenate([np.cos(ic_ang), -np.sin(ic_ang)], axis=0) / 64.0

    return {k: np.ascontiguousarray(v, dtype=np.float32) for k, v in t.items()}


TBL_SHAPES = {"s1w": (64, 128), "tt1": (128, FREE), "tts": (128, FREE),
              "s2w": (128, 96), "iaw": (96, 128),
              "itt1": (128, FREE), "itts": (128, FREE), "icw": (128, 64)}


def build_nc():
    nc = bass.Bass()
    x_ext = nc.declare_dram_parameter("x", [BLOC * C, N], f32, isOutput=False)
    out_ext = nc.declare_dram_parameter("out", [BLOC * C, N], f32, isOutput=True)
    wext = {}
    for nm in ["w1_re", "w1_im", "w2_re", "w2_im"]:
        wext[nm] = nc.declare_dram_parameter(nm, [NB, BLK, BLK], f32, isOutput=False)
    for nm in ["b1_re", "b1_im", "b2_re", "b2_im"]:
        wext[nm] = nc.declare_dram_parameter(nm, [NB, BLK], f32, isOutput=False)
    text = {nm: nc.declare_dram_parameter(nm, list(shp), f32, isOutput=False)
            for nm, shp in TBL_SHAPES.items()}

    with tile.TileContext(nc) as tc:
        with tc.tile_pool(name="cst", bufs=1) as cst, \
             tc.tile_pool(name="work", bufs=1) as pool, \
             tc.tile_pool(name="psbig", bufs=2, space="PSUM") as psB, \
             tc.tile_pool(name="psa", bufs=2, space="PSUM") as psA, \
             tc.tile_pool(name="psb", bufs=2, space="PSUM") as psD:

            # ---- constants ----
            ct = {}
            for nm in TBL_SHAPES:
                ct[nm] = cst.tile(list(TBL_SHAPES[nm]), bf16, tag=nm, name=nm)
                nc.gpsimd.dma_start(out=ct[nm][:], in_=text[nm][:])
            wt = {}
            for nm, src in [("w1re", "w1_re"), ("w1im", "w1_im"),
                            ("w2re", "w2_re"), ("w2im", "w2_im")]:
                wt[nm] = cst.tile([BLK, NB, BLK], bf16, tag=nm, name=nm)
                nc.gpsimd.dma_start(out=wt[nm][:], in_=wext[src].rearrange("b c o -> c b o"))
            w1imN = cst.tile([BLK, NB, BLK], bf16, tag="w1imN")
            nc.vector.tensor_scalar_mul(w1imN[:], wt["w1im"][:], -1.0)
            w2imN = cst.tile([BLK, NB, BLK], bf16, tag="w2imN")
            nc.vector.tensor_scalar_mul(w2imN[:], wt["w2im"][:], -1.0)
            bia = {}
            for nm, src in [("b1re", "b1_re"), ("b1im", "b1_im"),
                            ("b2re", "b2_re"), ("b2im", "b2_im")]:
                bia[nm] = cst.tile([BLK, NB], f32, tag=nm, name=nm)
                nc.gpsimd.dma_start(out=bia[nm][:], in_=wext[src].rearrange("b c -> c b"))
            b2v = {}
            for nm in ["b2re", "b2im"]:
                m = cst.tile([BLK, NB], f32, tag=f"{nm}_m", name=f"{nm}_m")   # b2 - lam
                nc.vector.tensor_scalar_add(m[:], bia[nm][:], -LAM)
                n_ = cst.tile([BLK, NB], f32, tag=f"{nm}_n", name=f"{nm}_n")  # -b2 - lam
                nc.vector.tensor_scalar(out=n_[:], in0=bia[nm][:], scalar1=-1.0,
                                        scalar2=-LAM, op0=MUL, op1=ADD)
                b2v[nm] = (m, n_)

            dma_engs = [nc.sync, nc.scalar, nc.gpsimd]

            def xpose(in_ap, out_t, P, M, Q, tag, eng=None, qoff=0):
                """in (P p; M, Q f) -> out (Q p; M, P f). P, Q mult of 32."""
                st = pool.tile([P, M, Q], bf16, tag=tag, name=tag)
                nc.vector.transpose(out=st[:], in_=in_ap)
                for pb in range(P // 32):
                    for qb in range(Q // 32):
                        xpose.rr = getattr(xpose, "rr", 0) + 1
                        e = dma_engs[xpose.rr % 3]
                        e.dma_start(
                            out=out_t[qoff + qb * 32: qoff + qb * 32 + 32, :,
                                      pb * 32: pb * 32 + 32],
                            in_=st[pb * 32: pb * 32 + 32, :, qb * 32: qb * 32 + 32])

            for u in range(N_UNITS):
                blk = u % NB
                bs = slice(blk, blk + 1)
                rows = slice(u * BLK, (u + 1) * BLK)
                xr = x_ext[rows].rearrange("c (a b) -> a c b", a=N1)

                # ---- P1: load, S1, fwd twiddle ----
                xt = pool.tile([64, BLK, N2], bf16, tag=f"xt{u % 2}")
                nc.gpsimd.dma_start(out=xt[:], in_=xr)
                xtf = xt[:].rearrange("p c b -> p (c b)")
                bduo = pool.tile([64, BLK, 128], bf16, tag="duo")
                for j in range(FREE // TCH):
                    a_ps = psB.tile([128, TCH], f32, tag="ps128")
                    for m in range(TCH // CH):
                        pc = slice(m * CH, (m + 1) * CH)
                        col = slice(j * TCH + m * CH, j * TCH + (m + 1) * CH)
                        nc.tensor.matmul(out=a_ps[:, pc], lhsT=ct["s1w"][:],
                                         rhs=xtf[:, col], start=True, stop=True)
                    sl = slice(j * TCH, (j + 1) * TCH)
                    abf = pool.tile([128, TCH], bf16, tag=f"abf{u % 2}{j % 2}")
                    nc.scalar.activation(out=abf[:], in_=a_ps[:], func=COPY)
                    uu = pool.tile([128, TCH], bf16, tag=f"uuF{u % 2}")
                    vv = pool.tile([128, TCH], bf16, tag=f"vvF{u % 2}")
                    nc.vector.tensor_tensor(out=uu[:], in0=abf[:], in1=ct["tt1"][:, sl], op=MUL)
                    nc.vector.tensor_tensor(out=vv[0:64, :], in0=abf[64:128, :],
                                            in1=ct["tts"][64:128, sl], op=MUL)
                    nc.vector.tensor_tensor(out=vv[64:128, :], in0=abf[0:64, :],
                                            in1=ct["tts"][0:64, sl], op=MUL)
                    cs = slice((j * TCH) // N2, ((j + 1) * TCH) // N2)
                    bre = bduo[:, cs, 0:64]
                    bim = bduo[:, cs, 64:128]
                    nc.vector.tensor_tensor(out=bre, in0=uu[0:64, :], in1=vv[0:64, :], op=ADD)
                    nc.vector.tensor_tensor(out=bim, in0=uu[64:128, :], in1=vv[64:128, :], op=ADD)

                def dump64(t):  # (64, 96, 64)-shaped bf16 tile -> out rows
                    nc.gpsimd.dma_start(out=out_ext[rows].rearrange("c (a b) -> a c b", a=N1), in_=t)

                def dump96(t):  # (96, X>=4096 flat) bf16 tile -> out rows
                    nc.gpsimd.dma_start(out=out_ext[rows], in_=t)

                if STOP_AFTER == "xt":
                    dump64(xt[:]); continue
                if STOP_AFTER == "bduo":
                    dump64(bduo[:, :, 0:64]); continue
                if STOP_AFTER == "bduo_im":
                    dump64(bduo[:, :, 64:128]); continue

                # ---- T1 (re/im halves; DVE on one half overlaps DMAs of other) ----
                bt = pool.tile([128, BLK, 64], bf16, tag=f"bt{u % 2}")
                xpose(bduo[:, :, 0:64], bt, 64, BLK, 64, "stA", eng=nc.sync, qoff=0)
                xpose(bduo[:, :, 64:128], bt, 64, BLK, 64, "stB", eng=nc.scalar, qoff=64)

                if STOP_AFTER == "bt":
                    dump64(bt[0:64]); continue
                if STOP_AFTER == "bt_im":
                    dump64(bt[64:128]); continue

                # ---- S2 + evict: (k1,c)-ordered chunks, contiguous act dst ----
                xf = pool.tile([96, 64, BLK], bf16, tag="mid1")  # (k2x; k1, c)
                SCH = 4 * BLK  # 4 k1 x 96 c
                for j in range(FREE // SCH):
                    s_ps = psA.tile([96, SCH], f32, tag="psa")
                    rhs = bt[:, :, j * 4:(j + 1) * 4].rearrange("p c k -> p k c")
                    nc.tensor.matmul(out=s_ps[:], lhsT=ct["s2w"][:],
                                     rhs=rhs, start=True, stop=True)
                    nc.scalar.activation(out=xf[:, j * 4:(j + 1) * 4, :], in_=s_ps[:],
                                         func=COPY)

                if STOP_AFTER == "xf":
                    dump96(xf[:].rearrange("p a b -> p (a b)")[:, 0:4096]); continue

                # ---- T2 (direct contiguous read) ----
                xft = pool.tile([96, 64, 96], bf16, tag="mid2")  # (c; k1, k2x96)
                xpose(xf[:], xft, 96, 64, 96, "stA", eng=nc.gpsimd)

                if STOP_AFTER == "xft":
                    dump96(xft[:].rearrange("p a b -> p (a b)")[:, 0:4096]); continue

                # ---- MLP ----
                ot = pool.tile([96, 64, 96], bf16, tag="mid1")
                for s in range(64 // SLAB):
                    ksl = slice(s * SLAB, (s + 1) * SLAB)
                    rre = xft[:, ksl, 0:33]
                    rim = xft[:, ksl, 33:66]
                    hre_ps = psA.tile([96, SLAB * 33], f32, tag="psa")
                    him_ps = psD.tile([96, SLAB * 33], f32, tag="psb")
                    nc.tensor.matmul(out=hre_ps[:], lhsT=wt["w1re"][:, blk], rhs=rre,
                                     start=True, stop=False, skip_group_check=True)
                    nc.tensor.matmul(out=him_ps[:], lhsT=wt["w1re"][:, blk], rhs=rim,
                                     start=True, stop=False, skip_group_check=True)
                    nc.tensor.matmul(out=hre_ps[:], lhsT=w1imN[:, blk], rhs=rim,
                                     start=False, stop=True, skip_group_check=True)
                    nc.tensor.matmul(out=him_ps[:], lhsT=wt["w1im"][:, blk], rhs=rre,
                                     start=False, stop=True, skip_group_check=True)
                    hre = pool.tile([96, SLAB * 33], bf16, tag=f"hre_s{s % 2}")
                    him = pool.tile([96, SLAB * 33], bf16, tag=f"him_s{s % 2}")
                    nc.scalar.activation(out=hre[:], in_=hre_ps[:], func=RELU,
                                         bias=bia["b1re"][:, bs], scale=1.0)
                    nc.scalar.activation(out=him[:], in_=him_ps[:], func=RELU,
                                         bias=bia["b1im"][:, bs], scale=1.0)
                    ore_ps = psA.tile([96, SLAB * 33], f32, tag="psa")
                    oim_ps = psD.tile([96, SLAB * 33], f32, tag="psb")
                    nc.tensor.matmul(out=ore_ps[:], lhsT=wt["w2re"][:, blk], rhs=hre[:],
                                     start=True, stop=False, skip_group_check=True)
                    nc.tensor.matmul(out=oim_ps[:], lhsT=wt["w2re"][:, blk], rhs=him[:],
                                     start=True, stop=False, skip_group_check=True)
                    nc.tensor.matmul(out=ore_ps[:], lhsT=w2imN[:, blk], rhs=him[:],
                                     start=False, stop=True, skip_group_check=True)
                    nc.tensor.matmul(out=oim_ps[:], lhsT=wt["w2im"][:, blk], rhs=hre[:],
                                     start=False, stop=True, skip_group_check=True)
                    for ps_, nm, qoff in [(ore_ps, "b2re", 0), (oim_ps, "b2im", 33)]:
                        bm, bn = b2v[nm]
                        t1_ = pool.tile([96, SLAB * 33], bf16, tag=f"ss1{s % 2}")
                        t2_ = pool.tile([96, SLAB * 33], bf16, tag=f"ss2{s % 2}")
                        nc.scalar.activation(out=t1_[:], in_=ps_[:], func=RELU,
                                             bias=bm[:, bs], scale=1.0)
                        nc.scalar.activation(out=t2_[:], in_=ps_[:], func=RELU,
                                             bias=bn[:, bs], scale=-1.0)
                        dst = ot[:, ksl, qoff:qoff + 33]
                        nc.vector.tensor_tensor(out=dst, in0=t1_[:], in1=t2_[:], op=SUB)

                # fixups: DC & Nyquist bins g=1 (halve); junk bins (k2=32,k1>0) zero
                for col in (0, 33, 32, 65):
                    nc.vector.tensor_scalar_mul(ot[:, 0, col:col + 1], ot[:, 0, col:col + 1], 0.5)
                nc.vector.memset(ot[:, 1:64, 32:33], 0.0)
                nc.vector.memset(ot[:, 1:64, 65:66], 0.0)

                if STOP_AFTER == "ot":
                    dump96(ot[:].rearrange("p a b -> p (a b)")[:, 0:4096]); continue

                # ---- T3 ----
                ott = pool.tile([96, 64, 96], bf16, tag="mid2")
                xpose(ot[:], ott, 96, 64, 96, "stB", eng=nc.gpsimd)

                if STOP_AFTER == "ott":
                    dump96(ott[:].rearrange("p a b -> p (a b)")[:, 0:4096]); continue

                # ---- iA + inverse twiddle ((c,k1)-ordered, contiguous dst) ----
                gduo = pool.tile([64, BLK, 128], bf16, tag="duo")  # (n2; c, k1pair)
                for j in range(FREE // ICH):
                    e_ps = psD.tile([128, ICH], f32, tag="psb")
                    rhs = ott[:, j * 4:(j + 1) * 4, :].rearrange("p k c -> p c k")
                    nc.tensor.matmul(out=e_ps[:], lhsT=ct["iaw"][:],
                                     rhs=rhs, start=True, stop=True)
                    sl = slice(j * ICH, (j + 1) * ICH)
                    ebf = pool.tile([128, ICH], bf16, tag=f"abf{u % 2}{j % 2}")
                    nc.scalar.activation(out=ebf[:], in_=e_ps[:], func=COPY)
                    uu = pool.tile([128, ICH], bf16, tag=f"uuF{u % 2}")
                    vv = pool.tile([128, ICH], bf16, tag=f"vvF{u % 2}")
                    nc.vector.tensor_tensor(out=uu[:], in0=ebf[:], in1=ct["itt1"][:, sl], op=MUL)
                    nc.vector.tensor_tensor(out=vv[0:64, :], in0=ebf[64:128, :],
                                            in1=ct["itts"][64:128, sl], op=MUL)
                    nc.vector.tensor_tensor(out=vv[64:128, :], in0=ebf[0:64, :],
                                            in1=ct["itts"][0:64, sl], op=MUL)
                    k0 = (j * ICH) // BLK
                    kn = ICH // BLK
                    nc.vector.tensor_tensor(out=gduo[:, :, k0:k0 + kn], in0=uu[0:64, :], in1=vv[0:64, :], op=ADD)
                    nc.vector.tensor_tensor(out=gduo[:, :, 64 + k0:64 + k0 + kn], in0=uu[64:128, :], in1=vv[64:128, :], op=ADD)

                if STOP_AFTER == "gduo":
                    dump64(gduo[:, :, 0:64]); continue

                # ---- T4 (re/im halves, direct contiguous reads) ----
                gt = pool.tile([128, BLK, 64], bf16, tag=f"bt{u % 2}")
                xpose(gduo[:, :, 0:64], gt, 64, BLK, 64, "stA", eng=nc.sync, qoff=0)
                xpose(gduo[:, :, 64:128], gt, 64, BLK, 64, "stB", eng=nc.scalar, qoff=64)

                if STOP_AFTER == "gt":
                    dump64(gt[0:64]); continue

                # ---- iC + residual (from xt) + store ----
                ybf = pool.tile([64, BLK, N2], bf16, tag="mid2")
                gtf = gt[:].rearrange("p c k -> p (c k)")
                xresf = xtf
                ybff = ybf[:].rearrange("p c b -> p (c b)")
                for j in range(FREE // CH):
                    y_ps = psA.tile([64, CH], f32, tag="psa")
                    nc.tensor.matmul(out=y_ps[:], lhsT=ct["icw"][:],
                                     rhs=gtf[:, j * CH:(j + 1) * CH], start=True, stop=True)
                    nc.vector.tensor_tensor(out=ybff[:, j * CH:(j + 1) * CH],
                                            in0=y_ps[:], in1=xresf[:, j * CH:(j + 1) * CH], op=ADD)
                orow = out_ext[rows].rearrange("c (a b) -> a c b", a=N1)
                nc.gpsimd.dma_start(out=orow, in_=ybf[:])

    return nc


_CACHE = {}


def kernel(**inputs):
    x = np.asarray(inputs["x"], dtype=np.float32)
    tables = make_tables()
    if "nc" not in _CACHE:
        _CACHE["nc"] = build_nc()
    nc = _CACHE["nc"]
    xs = x.reshape(NCORES, BLOC * C, N)
    wkeys = ["w1_re", "w1_im", "w2_re", "w2_im", "b1_re", "b1_im", "b2_re", "b2_im"]
    in_maps = []
    for cix in range(NCORES):
        m = {"x": np.ascontiguousarray(xs[cix])}
        for k in wkeys:
            m[k] = np.asarray(inputs[k], dtype=np.float32)
        m.update(tables)
        in_maps.append(m)
    res = run_bass_kernel_spmd(nc, in_maps, core_ids=list(range(NCORES)))
    _CACHE["last_res"] = res
    out = np.stack([res.results[i]["out"] for i in range(NCORES)], axis=0)
    return out.reshape(B, C, N)


if __name__ == "__main__":
    rng = np.random.default_rng(0)
    ins = {"x": rng.standard_normal((B, C, N), dtype=np.float32)}
    for k in ["w1_re", "w1_im", "w2_re", "w2_im"]:
        ins[k] = 0.02 * rng.random((NB, BLK, BLK), dtype=np.float32)
    for k in ["b1_re", "b1_im", "b2_re", "b2_im"]:
        ins[k] = 0.02 * rng.random((NB, BLK), dtype=np.float32)
    y = kernel(**ins)
    print("kernel ran, out shape", y.shape)



# revision 35
# speedup vs baseline: 1.0507x; 1.0351x over previous
"""AdaptiveFourier1d TRN2 kernel: y = irfft(MLP_freq(rfft(x))) + x.

Data-parallel over batch: 8 cores x 2 batch elements each. Per core 16
units of (96 ch, 4096). Radix-64^2 Cooley-Tukey FFT on TensorE; transposes
via DVE StreamTranspose (32x32 blocks) + block-permute DMAs; twiddles as
mixed-base DVE ops reading PSUM; MLP contracts channels.

Self-contained: all shapes hardcoded; constant tables passed as extra
dram parameters computed here with numpy.
"""
import os
import sys

sys.path.insert(0, "/opt/trn_rl_repo")

import numpy as np
import ml_dtypes

import concourse.bass as bass
import concourse.mybir as mybir
import concourse.tile as tile
from concourse.vector_clock import ScopedClock
from concourse.bass_utils import run_bass_kernel_spmd

bf16 = mybir.dt.bfloat16
f32 = mybir.dt.float32
RELU = mybir.ActivationFunctionType.Relu
COPY = mybir.ActivationFunctionType.Copy
MUL = mybir.AluOpType.mult
ADD = mybir.AluOpType.add
SUB = mybir.AluOpType.subtract

B, C, N = 16, 768, 4096
NB, BLK = 8, 96
N1 = N2 = 64
K2X = 33
LAM = 0.01
NCORES = 8
BLOC = B // NCORES
UNITS = BLOC * NB
FREE = BLK * N2          # 6144
CH = 512
TCH = 1024               # fwd twiddle chunk: 16 c x 64 n2
ICH = 384                # inv twiddle chunk: 4 k1 x 96 c'
SLAB = 8                 # MLP k1 slab

N_UNITS = int(os.environ.get("N_UNITS", UNITS))
STOP_AFTER = os.environ.get("STOP_AFTER", "")


def _patched_drain_and_barrier(self, tick_clock, wait_clock):
    nc = self.nc
    nops = [nc.sync.nop(nofuse=True) for _ in range(64)]
    drain_inst = nc.sync.drain()
    wait_clock.add_sem_waits(
        drain_inst.ins, ScopedClock({None: tick_clock.global_clock})
    )
    si = drain_inst.ins.sync_info
    waits = list(si.on_wait) if si and si.on_wait else []
    if len(waits) > 1:
        si.on_wait = [waits[0]]
        for w, nop in zip(waits[1:], nops):
            nsi = nop.ins.sync_info
            if nsi is None:
                nop.ins.sync_info = mybir.SyncInfo(on_wait=[w], on_update=[])
            else:
                nsi.on_wait = [w]
    nc.all_engine_barrier()
    popped = nc._tile_sem_poison_stack.pop()
    assert popped is self._sem_poison
    nc.clear_and_free_semaphores(list(self.sems.allocated().values()))
    nc.all_engine_barrier()


tile.TileContext._drain_and_barrier = _patched_drain_and_barrier

# walrus in this toolchain rejects >2 sync waits per instruction; split extras
# onto same-engine nops committed just before the instruction.
MAXW = 1
_orig_commit = tile.TileContext._commit_instruction


def _commit_split(self, inst, lazy_reg_writes=True):
    si = getattr(inst, "sync_info", None)
    if (si is not None and si.on_wait and len(si.on_wait) > MAXW
            and inst.engine != mybir.EngineType.Unassigned):
        waits = list(si.on_wait)
        si.on_wait = waits[-MAXW:]
        extra = waits[:-MAXW]
        eng = self.nc.engines[inst.engine]
        for i in range(0, len(extra), MAXW):
            nop = eng.nop(nofuse=True)
            nop.ins.sync_info = mybir.SyncInfo(on_wait=extra[i:i + MAXW], on_update=[])
    return _orig_commit(self, inst, lazy_reg_writes)


tile.TileContext._commit_instruction = _commit_split


def make_tables():
    t = {}
    n1 = np.arange(N1)
    k1 = np.arange(N1)
    n2 = np.arange(N2)
    k2 = np.arange(K2X)

    a = 2 * np.pi * np.outer(n1, k1) / N1
    t["s1w"] = np.concatenate([np.cos(a), -np.sin(a)], axis=1) / 64.0  # (64,128)

    ang = 2 * np.pi * np.outer(k1, n2) / N          # T[k1, n2] = e^{-i ang}
    tt1 = np.tile(np.cos(ang), (1, BLK))            # (64, FREE) c-outer n2-inner
    tsn = np.tile(np.sin(ang), (1, BLK))
    t["tt1"] = np.concatenate([tt1, tt1], axis=0)   # (128, FREE) for full-width ops
    t["tts"] = np.concatenate([-tsn, tsn], axis=0)  # rows0:-sin (for bim), rows1:+sin (bre)

    o_ang = 2 * np.pi * np.outer(n2, k2) / N2       # Omega = e^{-i o_ang}
    Ore, Oim = np.cos(o_ang), -np.sin(o_ang)
    z = np.zeros((N2, 96 - 2 * K2X))
    top = np.concatenate([Ore, Oim, z], axis=1)
    bot = np.concatenate([-Oim, Ore, z], axis=1)
    t["s2w"] = np.concatenate([top, bot], axis=0)   # (128, 96)

    ia_ang = 2 * np.pi * np.outer(k2, n2) / N2      # e^{+i}
    iare = np.concatenate([2 * np.cos(ia_ang), 2 * np.sin(ia_ang)], axis=1)
    iaim = np.concatenate([-2 * np.sin(ia_ang), 2 * np.cos(ia_ang)], axis=1)
    t["iaw"] = np.concatenate([iare, iaim, np.zeros((96 - 2 * K2X, 128))], axis=0)

    it_ang = 2 * np.pi * np.outer(n2, k1) / N       # T'[n2',k1] = e^{+i}
    # chunk order: (k1-group-of-4 outer, c middle, k1-within inner)
    def ck(m):
        return np.broadcast_to(
            m.reshape(N1, 16, 1, 4), (N1, 16, BLK, 4)).reshape(N1, FREE)
    itt1 = ck(np.cos(it_ang))
    itsn = ck(np.sin(it_ang))
    t["itt1"] = np.concatenate([itt1, itt1], axis=0)
    t["itts"] = np.concatenate([itsn, -itsn], axis=0)  # rows0:+sin (gim), rows1:-sin (gre)

    ic_ang = 2 * np.pi * np.outer(k1, n1) / N1      # e^{+i}
    t["icw"] = np.concatenate([np.cos(ic_ang), -np.sin(ic_ang)], axis=0) / 64.0

    return {k: np.ascontiguousarray(v, dtype=np.float32) for k, v in t.items()}


TBL_SHAPES = {"s1w": (64, 128), "tt1": (128, FREE), "tts": (128, FREE),
              "s2w": (128, 96), "iaw": (96, 128),
              "itt1": (128, FREE), "itts": (128, FREE), "icw": (128, 64)}


def build_nc():
    nc = bass.Bass()
    x_ext = nc.declare_dram_parameter("x", [BLOC * C, N], f32, isOutput=False)
    out_ext = nc.declare_dram_parameter("out", [BLOC * C, N], f32, isOutput=True)
    wext = {}
    for nm in ["w1_re", "w1_im", "w2_re", "w2_im"]:
        wext[nm] = nc.declare_dram_parameter(nm, [NB, BLK, BLK], f32, isOutput=False)
    for nm in ["b1_re", "b1_im", "b2_re", "b2_im"]:
        wext[nm] = nc.declare_dram_parameter(nm, [NB, BLK], f32, isOutput=False)
    text = {nm: nc.declare_dram_parameter(nm, list(shp), f32, isOutput=False)
            for nm, shp in TBL_SHAPES.items()}

    with tile.TileContext(nc) as tc:
        with tc.tile_pool(name="cst", bufs=1) as cst, \
             tc.tile_pool(name="work", bufs=1) as pool, \
             tc.tile_pool(name="psbig", bufs=2, space="PSUM") as psB, \
             tc.tile_pool(name="psa", bufs=2, space="PSUM") as psA, \
             tc.tile_pool(name="psb", bufs=2, space="PSUM") as psD:

            # ---- constants ----
            ct = {}
            for nm in TBL_SHAPES:
                ct[nm] = cst.tile(list(TBL_SHAPES[nm]), bf16, tag=nm, name=nm)
                nc.gpsimd.dma_start(out=ct[nm][:], in_=text[nm][:])
            wt = {}
            for nm, src in [("w1re", "w1_re"), ("w1im", "w1_im"),
                            ("w2re", "w2_re"), ("w2im", "w2_im")]:
                wt[nm] = cst.tile([BLK, NB, BLK], bf16, tag=nm, name=nm)
                nc.gpsimd.dma_start(out=wt[nm][:], in_=wext[src].rearrange("b c o -> c b o"))
            w1imN = cst.tile([BLK, NB, BLK], bf16, tag="w1imN")
            nc.vector.tensor_scalar_mul(w1imN[:], wt["w1im"][:], -1.0)
            w2imN = cst.tile([BLK, NB, BLK], bf16, tag="w2imN")
            nc.vector.tensor_scalar_mul(w2imN[:], wt["w2im"][:], -1.0)
            bia = {}
            for nm, src in [("b1re", "b1_re"), ("b1im", "b1_im"),
                            ("b2re", "b2_re"), ("b2im", "b2_im")]:
                bia[nm] = cst.tile([BLK, NB], f32, tag=nm, name=nm)
                nc.gpsimd.dma_start(out=bia[nm][:], in_=wext[src].rearrange("b c -> c b"))
            b2v = {}
            for nm in ["b2re", "b2im"]:
                m = cst.tile([BLK, NB], f32, tag=f"{nm}_m", name=f"{nm}_m")   # b2 - lam
                nc.vector.tensor_scalar_add(m[:], bia[nm][:], -LAM)
                n_ = cst.tile([BLK, NB], f32, tag=f"{nm}_n", name=f"{nm}_n")  # -b2 - lam
                nc.vector.tensor_scalar(out=n_[:], in0=bia[nm][:], scalar1=-1.0,
                                        scalar2=-LAM, op0=MUL, op1=ADD)
                b2v[nm] = (m, n_)

            dma_engs = [nc.sync, nc.scalar, nc.gpsimd]

            def xpose(in_ap, out_t, P, M, Q, tag, eng=None, qoff=0):
                """in (P p; M, Q f) -> out (Q p; M, P f). P, Q mult of 32."""
                st = pool.tile([P, M, Q], bf16, tag=tag, name=tag)
                nc.vector.transpose(out=st[:], in_=in_ap)
                for pb in range(P // 32):
                    for qb in range(Q // 32):
                        xpose.rr = getattr(xpose, "rr", 0) + 1
                        e = dma_engs[xpose.rr % 3]
                        e.dma_start(
                            out=out_t[qoff + qb * 32: qoff + qb * 32 + 32, :,
                                      pb * 32: pb * 32 + 32],
                            in_=st[pb * 32: pb * 32 + 32, :, qb * 32: qb * 32 + 32])

            for u in range(N_UNITS):
                blk = u % NB
                bs = slice(blk, blk + 1)
                rows = slice(u * BLK, (u + 1) * BLK)
                xr = x_ext[rows].rearrange("c (a b) -> a c b", a=N1)

                # ---- P1: load, S1, fwd twiddle ----
                xt = pool.tile([64, BLK, N2], bf16, tag=f"xt{u % 2}")
                nc.gpsimd.dma_start(out=xt[:], in_=xr)
                xtf = xt[:].rearrange("p c b -> p (c b)")
                bduo = pool.tile([64, BLK, 128], bf16, tag="duo")
                for j in range(FREE // TCH):
                    a_ps = psB.tile([128, TCH], f32, tag="ps128")
                    for m in range(TCH // CH):
                        pc = slice(m * CH, (m + 1) * CH)
                        col = slice(j * TCH + m * CH, j * TCH + (m + 1) * CH)
                        nc.tensor.matmul(out=a_ps[:, pc], lhsT=ct["s1w"][:],
                                         rhs=xtf[:, col], start=True, stop=True)
                    sl = slice(j * TCH, (j + 1) * TCH)
                    abf = pool.tile([128, TCH], bf16, tag=f"abf{u % 2}{j % 2}")
                    nc.scalar.activation(out=abf[:], in_=a_ps[:], func=COPY)
                    uu = pool.tile([128, TCH], bf16, tag=f"uuF{u % 2}")
                    vv = pool.tile([128, TCH], bf16, tag=f"vvF{u % 2}")
                    nc.vector.tensor_tensor(out=uu[:], in0=abf[:], in1=ct["tt1"][:, sl], op=MUL)
                    nc.vector.tensor_tensor(out=vv[0:64, :], in0=abf[64:128, :],
                                            in1=ct["tts"][64:128, sl], op=MUL)
                    nc.vector.tensor_tensor(out=vv[64:128, :], in0=abf[0:64, :],
                                            in1=ct["tts"][0:64, sl], op=MUL)
                    cs = slice((j * TCH) // N2, ((j + 1) * TCH) // N2)
                    bre = bduo[:, cs, 0:64]
                    bim = bduo[:, cs, 64:128]
                    nc.vector.tensor_tensor(out=bre, in0=uu[0:64, :], in1=vv[0:64, :], op=ADD)
                    nc.vector.tensor_tensor(out=bim, in0=uu[64:128, :], in1=vv[64:128, :], op=ADD)

                def dump64(t):  # (64, 96, 64)-shaped bf16 tile -> out rows
                    nc.gpsimd.dma_start(out=out_ext[rows].rearrange("c (a b) -> a c b", a=N1), in_=t)

                def dump96(t):  # (96, X>=4096 flat) bf16 tile -> out rows
                    nc.gpsimd.dma_start(out=out_ext[rows], in_=t)

                if STOP_AFTER == "xt":
                    dump64(xt[:]); continue
                if STOP_AFTER == "bduo":
                    dump64(bduo[:, :, 0:64]); continue
                if STOP_AFTER == "bduo_im":
                    dump64(bduo[:, :, 64:128]); continue

                # ---- T1 (re/im halves; DVE on one half overlaps DMAs of other) ----
                bt = pool.tile([128, BLK, 64], bf16, tag=f"bt{u % 2}")
                xpose(bduo[:, :, 0:64], bt, 64, BLK, 64, "stA", eng=nc.sync, qoff=0)
                xpose(bduo[:, :, 64:128], bt, 64, BLK, 64, "stB", eng=nc.scalar, qoff=64)

                if STOP_AFTER == "bt":
                    dump64(bt[0:64]); continue
                if STOP_AFTER == "bt_im":
                    dump64(bt[64:128]); continue

                # ---- S2 + evict: (k1,c)-ordered chunks, contiguous act dst ----
                xf = pool.tile([96, 64, BLK], bf16, tag="mid1")  # (k2x; k1, c)
                SCH = 4 * BLK  # 4 k1 x 96 c
                for j in range(FREE // SCH):
                    s_ps = psA.tile([96, SCH], f32, tag="psa")
                    rhs = bt[:, :, j * 4:(j + 1) * 4].rearrange("p c k -> p k c")
                    nc.tensor.matmul(out=s_ps[:], lhsT=ct["s2w"][:],
                                     rhs=rhs, start=True, stop=True)
                    nc.scalar.activation(out=xf[:, j * 4:(j + 1) * 4, :], in_=s_ps[:],
                                         func=COPY)

                if STOP_AFTER == "xf":
                    dump96(xf[:].rearrange("p a b -> p (a b)")[:, 0:4096]); continue

                # ---- T2 (direct contiguous read) ----
                xft = pool.tile([96, 64, 96], bf16, tag="mid2")  # (c; k1, k2x96)
                xpose(xf[:], xft, 96, 64, 96, "stA", eng=nc.gpsimd)

                if STOP_AFTER == "xft":
                    dump96(xft[:].rearrange("p a b -> p (a b)")[:, 0:4096]); continue

                # ---- MLP ----
                ot = pool.tile([96, 64, 96], bf16, tag="mid1")
                for s in range(64 // SLAB):
                    ksl = slice(s * SLAB, (s + 1) * SLAB)
                    rre = xft[:, ksl, 0:33]
                    rim = xft[:, ksl, 33:66]
                    hre_ps = psA.tile([96, SLAB * 33], f32, tag="psa")
                    him_ps = psD.tile([96, SLAB * 33], f32, tag="psb")
                    nc.tensor.matmul(out=hre_ps[:], lhsT=wt["w1re"][:, blk], rhs=rre,
                                     start=True, stop=False, skip_group_check=True)
                    nc.tensor.matmul(out=him_ps[:], lhsT=wt["w1re"][:, blk], rhs=rim,
                                     start=True, stop=False, skip_group_check=True)
                    nc.tensor.matmul(out=hre_ps[:], lhsT=w1imN[:, blk], rhs=rim,
                                     start=False, stop=True, skip_group_check=True)
                    nc.tensor.matmul(out=him_ps[:], lhsT=wt["w1im"][:, blk], rhs=rre,
                                     start=False, stop=True, skip_group_check=True)
                    hre = pool.tile([96, SLAB * 33], bf16, tag=f"hre_s{s % 2}")
                    him = pool.tile([96, SLAB * 33], bf16, tag=f"him_s{s % 2}")
                    nc.scalar.activation(out=hre[:], in_=hre_ps[:], func=RELU,
                                         bias=bia["b1re"][:, bs], scale=1.0)
                    nc.scalar.activation(out=him[:], in_=him_ps[:], func=RELU,
                                         bias=bia["b1im"][:, bs], scale=1.0)
                    ore_ps = psA.tile([96, SLAB * 33], f32, tag="psa")
                    oim_ps = psD.tile([96, SLAB * 33], f32, tag="psb")
                    nc.tensor.matmul(out=ore_ps[:], lhsT=wt["w2re"][:, blk], rhs=hre[:],
                                     start=True, stop=False, skip_group_check=True)
                    nc.tensor.matmul(out=oim_ps[:], lhsT=wt["w2re"][:, blk], rhs=him[:],
                                     start=True, stop=False, skip_group_check=True)
                    nc.tensor.matmul(out=ore_ps[:], lhsT=w2imN[:, blk], rhs=him[:],
                                     start=False, stop=True, skip_group_check=True)
                    nc.tensor.matmul(out=oim_ps[:], lhsT=wt["w2im"][:, blk], rhs=hre[:],
                                     start=False, stop=True, skip_group_check=True)
                    for ps_, nm, qoff in [(ore_ps, "b2re", 0), (oim_ps, "b2im", 33)]:
                        bm, bn = b2v[nm]
                        t1_ = pool.tile([96, SLAB * 33], bf16, tag=f"ss1{s % 2}")
                        t2_ = pool.tile([96, SLAB * 33], bf16, tag=f"ss2{s % 2}")
                        nc.scalar.activation(out=t1_[:], in_=ps_[:], func=RELU,
                                             bias=bm[:, bs], scale=1.0)
                        nc.scalar.activation(out=t2_[:], in_=ps_[:], func=RELU,
                                             bias=bn[:, bs], scale=-1.0)
                        dst = ot[:, ksl, qoff:qoff + 33]
                        nc.vector.tensor_tensor(out=dst, in0=t1_[:], in1=t2_[:], op=SUB)

                # fixups: DC & Nyquist bins g=1 (halve); junk bins (k2=32,k1>0) zero
                for col in (0, 33, 32, 65):
                    nc.vector.tensor_scalar_mul(ot[:, 0, col:col + 1], ot[:, 0, col:col + 1], 0.5)
                nc.vector.memset(ot[:, 1:64, 32:33], 0.0)
                nc.vector.memset(ot[:, 1:64, 65:66], 0.0)

                if STOP_AFTER == "ot":
                    dump96(ot[:].rearrange("p a b -> p (a b)")[:, 0:4096]); continue

                # ---- T3 ----
                ott = pool.tile([96, 64, 96], bf16, tag="mid2")
                xpose(ot[:], ott, 96, 64, 96, "stB", eng=nc.gpsimd)

                if STOP_AFTER == "ott":
                    dump96(ott[:].rearrange("p a b -> p (a b)")[:, 0:4096]); continue

                # ---- iA + inverse twiddle ((c,k1)-ordered, chunk pairs) ----
                gduo = pool.tile([64, BLK, 128], bf16, tag="duo")  # (n2; c, k1pair)
                ICH2 = 2 * ICH
                for j2 in range(FREE // ICH2):
                    ebf = pool.tile([128, ICH2], bf16, tag=f"abf{u % 2}{j2 % 2}")
                    for h in range(2):
                        j = 2 * j2 + h
                        e_ps = psD.tile([128, ICH], f32, tag="psb")
                        rhs = ott[:, j * 4:(j + 1) * 4, :].rearrange("p k c -> p c k")
                        nc.tensor.matmul(out=e_ps[:], lhsT=ct["iaw"][:],
                                         rhs=rhs, start=True, stop=True)
                        nc.scalar.activation(out=ebf[:, h * ICH:(h + 1) * ICH],
                                             in_=e_ps[:], func=COPY)
                    sl = slice(j2 * ICH2, (j2 + 1) * ICH2)
                    uu = pool.tile([128, ICH2], bf16, tag=f"uuF{u % 2}")
                    vv = pool.tile([128, ICH2], bf16, tag=f"vvF{u % 2}")
                    nc.vector.tensor_tensor(out=uu[:], in0=ebf[:], in1=ct["itt1"][:, sl], op=MUL)
                    nc.vector.tensor_tensor(out=vv[0:64, :], in0=ebf[64:128, :],
                                            in1=ct["itts"][64:128, sl], op=MUL)
                    nc.vector.tensor_tensor(out=vv[64:128, :], in0=ebf[0:64, :],
                                            in1=ct["itts"][0:64, sl], op=MUL)
                    k0 = (j2 * ICH2) // BLK
                    gre = gduo[:, :, k0:k0 + 8].rearrange("p c (h k) -> p h c k", h=2)
                    gim = gduo[:, :, 64 + k0:64 + k0 + 8].rearrange("p c (h k) -> p h c k", h=2)
                    nc.vector.tensor_tensor(out=gre, in0=uu[0:64, :], in1=vv[0:64, :], op=ADD)
                    nc.vector.tensor_tensor(out=gim, in0=uu[64:128, :], in1=vv[64:128, :], op=ADD)

                if STOP_AFTER == "gduo":
                    dump64(gduo[:, :, 0:64]); continue

                # ---- T4 (re/im halves, direct contiguous reads) ----
                gt = pool.tile([128, BLK, 64], bf16, tag=f"bt{u % 2}")
                xpose(gduo[:, :, 0:64], gt, 64, BLK, 64, "stA", eng=nc.sync, qoff=0)
                xpose(gduo[:, :, 64:128], gt, 64, BLK, 64, "stB", eng=nc.scalar, qoff=64)

                if STOP_AFTER == "gt":
                    dump64(gt[0:64]); continue

                # ---- iC + residual (from xt) + store ----
                ybf = pool.tile([64, BLK, N2], bf16, tag="mid2")
                gtf = gt[:].rearrange("p c k -> p (c k)")
                xresf = xtf
                ybff = ybf[:].rearrange("p c b -> p (c b)")
                for j in range(FREE // CH):
                    y_ps = psA.tile([64, CH], f32, tag="psa")
                    nc.tensor.matmul(out=y_ps[:], lhsT=ct["icw"][:],
                                     rhs=gtf[:, j * CH:(j + 1) * CH], start=True, stop=True)
                    nc.vector.tensor_tensor(out=ybff[:, j * CH:(j + 1) * CH],
                                            in0=y_ps[:], in1=xresf[:, j * CH:(j + 1) * CH], op=ADD)
                orow = out_ext[rows].rearrange("c (a b) -> a c b", a=N1)
                nc.gpsimd.dma_start(out=orow, in_=ybf[:])

    return nc


_CACHE = {}


def kernel(**inputs):
    x = np.asarray(inputs["x"], dtype=np.float32)
    tables = make_tables()
    if "nc" not in _CACHE:
        _CACHE["nc"] = build_nc()
    nc = _CACHE["nc"]
    xs = x.reshape(NCORES, BLOC * C, N)
    wkeys = ["w1_re", "w1_im", "w2_re", "w2_im", "b1_re", "b1_im", "b2_re", "b2_im"]
    in_maps = []
    for cix in range(NCORES):
        m = {"x": np.ascontiguousarray(xs[cix])}
        for k in wkeys:
            m[k] = np.asarray(inputs[k], dtype=np.float32)
        m.update(tables)
        in_maps.append(m)
    res = run_bass_kernel_spmd(nc, in_maps, core_ids=list(range(NCORES)))
    _CACHE["last_res"] = res
    out = np.stack([res.results[i]["out"] for i in range(NCORES)], axis=0)
    return out.reshape(B, C, N)


if __name__ == "__main__":
    rng = np.random.default_rng(0)
    ins = {"x": rng.standard_normal((B, C, N), dtype=np.float32)}
    for k in ["w1_re", "w1_im", "w2_re", "w2_im"]:
        ins[k] = 0.02 * rng.random((NB, BLK, BLK), dtype=np.float32)
    for k in ["b1_re", "b1_im", "b2_re", "b2_im"]:
        ins[k] = 0.02 * rng.random((NB, BLK), dtype=np.float32)
    y = kernel(**ins)
    print("kernel ran, out shape", y.shape)

